# revision 8
# baseline (speedup 1.0000x reference)
"""DelayBuffer Trainium2 kernel.

Input:  embeddings [4, 4096, 1024] f32.
Output: [4, 4096, 6144] f32 — for each delay d in (1,2,4,8,16,32),
        out_d[t] = emb[t-d] if t >= d else emb[t], concatenated on the
        last axis.

Sharding: 8 cores = (batch b in 0..3) x (embed half h in 0..1). Each
core receives a contiguous [4096, 512] shard and produces [6, 4096*512]
(the six delayed copies of its shard). Host reassembles.

Kernel (pure DMA): stage the shard in SBUF with rows laid out 32 per
partition in row-major order (row r -> partition r//32, chunk r%32, so
partition-major iteration order equals row order). Then each delayed
copy is 3 SBUF->DRAM stores: bulk (rows 0..4063 -> d..4063+d), tail
(remaining rows from the last partition, clipped at 4096), head
(identity rows t < d).
"""

import numpy as np

import concourse.bass as bass
import concourse.tile as tile
from concourse import mybir
from concourse.bass_utils import run_bass_kernel_spmd

DELAYS = (1, 2, 4, 8, 16, 32)
B, S, D = 4, 4096, 1024
NCORES = 8
C = 512           # channels per core (half of D)
P = 128           # SBUF partitions
RPP = S // P      # rows per partition = 32
FREE = RPP * C    # floats per partition = 16384

_cached_nc = None


def _split_multi_waits(nc: bass.Bass) -> None:
    # This walrus version can encode only ONE sync-wait per instruction
    # (the TPB header's single EVENTS slot); codegen aborts with "Too many
    # sync wait commands" otherwise. The Tile kernel-tail drain waits on
    # every DMA sem lane, so split: hoist all but the last wait onto
    # fresh single-wait NoOps inserted just before the instruction on the
    # same engine queue.
    from concourse import mybir

    for f in nc.m.functions:
        for bb in f.blocks:
            new_insts = []
            for inst in bb.instructions:
                si = getattr(inst, "sync_info", None)
                if si is not None and si.on_wait and len(si.on_wait) > 1:
                    for w in si.on_wait[:-1]:
                        nop = mybir.InstNoOp(
                            name=nc.get_next_instruction_name(),
                            engine=inst.engine,
                        )
                        nop.sync_info = mybir.SyncInfo(on_wait=[w], on_update=[])
                        new_insts.append(nop)
                    si.on_wait = [si.on_wait[-1]]
                new_insts.append(inst)
            bb.instructions[:] = new_insts


def _build_program(reps: int = 1) -> bass.Bass:
    # reps > 1 repeats the whole kernel serially inside one NEFF (the
    # shared SBUF tile's WAR/WAW deps force rep i+1's load to wait for
    # rep i's stores) — used only for benchmarking, where the marginal
    # time between two rep counts cancels the multi-ms PJRT dispatch
    # overhead of this axon client.
    nc = bass.Bass()
    x = nc.declare_dram_parameter("x", [S, C], mybir.dt.float32, isOutput=False)
    y = nc.declare_dram_parameter(
        "y", [len(DELAYS), S * C], mybir.dt.float32, isOutput=True
    )
    with tile.TileContext(nc) as tc:
        with tc.tile_pool(name="sbuf", bufs=1) as pool:
            xt = pool.tile([P, FREE], mybir.dt.float32)
            for _ in range(reps):
                # Load: row r = RPP*p + n  ->  partition p, chunk n.
                nc.sync.dma_start(
                    out=xt[:], in_=x.rearrange("(p n) c -> p n c", p=P)
                )
                for k, d in enumerate(DELAYS):
                    yk = y[k]
                    # Bulk: rows 0..(P-1)*RPP-1 (partitions 0..P-2) -> rows d..
                    nbulk = (P - 1) * RPP * C
                    nc.sync.dma_start(
                        out=yk[d * C : d * C + nbulk], in_=xt[0 : P - 1, :]
                    )
                    # Tail: partition P-1 holds rows (P-1)*RPP..S-1; keep the
                    # first RPP-d rows, they land at rows (P-1)*RPP+d..S-1.
                    if d < RPP:
                        nc.sync.dma_start(
                            out=yk[((P - 1) * RPP + d) * C : S * C],
                            in_=xt[P - 1 : P, 0 : (RPP - d) * C],
                        )
                    # Head: rows t < d are identity.
                    nc.sync.dma_start(out=yk[0 : d * C], in_=xt[0:1, 0 : d * C])
    _split_multi_waits(nc)
    return nc


def kernel(embeddings: np.ndarray) -> np.ndarray:
    global _cached_nc
    embeddings = np.ascontiguousarray(embeddings, dtype=np.float32)
    assert embeddings.shape == (B, S, D)

    if _cached_nc is None:
        _cached_nc = _build_program()
    nc = _cached_nc

    # Shard: core c -> batch c//2, embed half c%2.
    in_maps = []
    for c in range(NCORES):
        b, h = divmod(c, 2)
        in_maps.append(
            {"x": np.ascontiguousarray(embeddings[b, :, h * C : (h + 1) * C])}
        )

    results = run_bass_kernel_spmd(nc, in_maps, list(range(NCORES))).results

    out = np.empty((B, S, len(DELAYS) * D), dtype=np.float32)
    for c in range(NCORES):
        b, h = divmod(c, 2)
        yk = results[c]["y"].reshape(len(DELAYS), S, C)
        for k in range(len(DELAYS)):
            out[b, :, k * D + h * C : k * D + (h + 1) * C] = yk[k]
    return out


# revision 10
# speedup vs baseline: 2.0837x; 2.0837x over previous
"""DelayBuffer Trainium2 kernel.

Input:  embeddings [4, 4096, 1024] f32.
Output: [4, 4096, 6144] f32 — for each delay d in (1,2,4,8,16,32),
        out_d[t] = emb[t-d] if t >= d else emb[t], concatenated on the
        last axis.

Sharding: 8 cores = (batch b in 0..3) x (embed half h in 0..1). Each
core receives a contiguous [4096, 512] shard and produces [6, 4096*512]
(the six delayed copies of its shard). Host reassembles.

Kernel (pure DMA): stage the shard in SBUF with rows laid out 32 per
partition in row-major order (row r -> partition r//32, chunk r%32, so
partition-major iteration order equals row order). Then each delayed
copy is 3 SBUF->DRAM stores: bulk (rows 0..4063 -> d..4063+d), tail
(remaining rows from the last partition, clipped at 4096), head
(identity rows t < d).
"""

import numpy as np

import concourse.bass as bass
import concourse.tile as tile
from concourse import mybir
from concourse.bass_utils import run_bass_kernel_spmd

DELAYS = (1, 2, 4, 8, 16, 32)
B, S, D = 4, 4096, 1024
NCORES = 8
C = 512           # channels per core (half of D)
P = 128           # SBUF partitions
RPP = S // P      # rows per partition = 32
FREE = RPP * C    # floats per partition = 16384

_cached_nc = None


def _split_multi_waits(nc: bass.Bass) -> None:
    # This walrus version can encode only ONE sync-wait per instruction
    # (the TPB header's single EVENTS slot); codegen aborts with "Too many
    # sync wait commands" otherwise. The Tile kernel-tail drain waits on
    # every DMA sem lane, so split: hoist all but the last wait onto
    # fresh single-wait NoOps inserted just before the instruction on the
    # same engine queue.
    from concourse import mybir

    for f in nc.m.functions:
        for bb in f.blocks:
            new_insts = []
            for inst in bb.instructions:
                si = getattr(inst, "sync_info", None)
                if si is not None and si.on_wait and len(si.on_wait) > 1:
                    for w in si.on_wait[:-1]:
                        nop = mybir.InstNoOp(
                            name=nc.get_next_instruction_name(),
                            engine=inst.engine,
                        )
                        nop.sync_info = mybir.SyncInfo(on_wait=[w], on_update=[])
                        new_insts.append(nop)
                    si.on_wait = [si.on_wait[-1]]
                new_insts.append(inst)
            bb.instructions[:] = new_insts


def _build_program(reps: int = 1, engine: str = "gpsimd") -> bass.Bass:
    # reps > 1 repeats the whole kernel serially inside one NEFF (the
    # shared SBUF tile's WAR/WAW deps force rep i+1's load to wait for
    # rep i's stores) — used only for benchmarking, where the marginal
    # time between two rep counts cancels the multi-ms PJRT dispatch
    # overhead of this axon client.
    nc = bass.Bass()
    x = nc.declare_dram_parameter("x", [S, C], mybir.dt.float32, isOutput=False)
    y = nc.declare_dram_parameter(
        "y", [len(DELAYS), S * C], mybir.dt.float32, isOutput=True
    )
    eng = getattr(nc, engine)
    with tile.TileContext(nc) as tc:
        with tc.tile_pool(name="sbuf", bufs=1) as pool:
            xt = pool.tile([P, FREE], mybir.dt.float32)
            for _ in range(reps):
                # Load: row r = RPP*p + n  ->  partition p, chunk n.
                eng.dma_start(
                    out=xt[:], in_=x.rearrange("(p n) c -> p n c", p=P)
                )
                for k, d in enumerate(DELAYS):
                    yk = y[k]
                    # Bulk: rows 0..(P-1)*RPP-1 (partitions 0..P-2) -> rows d..
                    nbulk = (P - 1) * RPP * C
                    eng.dma_start(
                        out=yk[d * C : d * C + nbulk], in_=xt[0 : P - 1, :]
                    )
                    # Tail: partition P-1 holds rows (P-1)*RPP..S-1; keep the
                    # first RPP-d rows, they land at rows (P-1)*RPP+d..S-1.
                    if d < RPP:
                        eng.dma_start(
                            out=yk[((P - 1) * RPP + d) * C : S * C],
                            in_=xt[P - 1 : P, 0 : (RPP - d) * C],
                        )
                    # Head: rows t < d are identity.
                    eng.dma_start(out=yk[0 : d * C], in_=xt[0:1, 0 : d * C])
    _split_multi_waits(nc)
    return nc


def kernel(embeddings: np.ndarray) -> np.ndarray:
    global _cached_nc
    embeddings = np.ascontiguousarray(embeddings, dtype=np.float32)
    assert embeddings.shape == (B, S, D)

    if _cached_nc is None:
        _cached_nc = _build_program()
    nc = _cached_nc

    # Shard: core c -> batch c//2, embed half c%2.
    in_maps = []
    for c in range(NCORES):
        b, h = divmod(c, 2)
        in_maps.append(
            {"x": np.ascontiguousarray(embeddings[b, :, h * C : (h + 1) * C])}
        )

    results = run_bass_kernel_spmd(nc, in_maps, list(range(NCORES))).results

    out = np.empty((B, S, len(DELAYS) * D), dtype=np.float32)
    for c in range(NCORES):
        b, h = divmod(c, 2)
        yk = results[c]["y"].reshape(len(DELAYS), S, C)
        for k in range(len(DELAYS)):
            out[b, :, k * D + h * C : k * D + (h + 1) * C] = yk[k]
    return out
